# revision 6
# baseline (speedup 1.0000x reference)
"""TRN2 Bass kernel for GQA attention (nn_Attention_19533511262498).

Tensor-parallel over heads across 8 NeuronCores: core c owns q-heads
[4c, 4c+4) and kv-head c (wq/wk/wv sharded on the head dim, wo on its
input dim). Each core computes a partial [S, DIM] output; the host sums
the 8 partials.

All matmuls run in float32r (tf32-like, ~11 mantissa bits, full PE rate
at moving-dim >= 256). Everything is kept in transposed ([feature, seq])
layouts so no on-device transposes are needed except a DRAM-bounce
re-read of V. Softmax skips the max-subtraction (scores are bounded by
construction) and normalization is applied to the attention output
before the wo projection.

RoPE trick: wq/wk rows are permuted on the host (even dims first, odd
dims second within each head) so the rotation becomes pure half-tile
elementwise ops; the permutation cancels in q.k dot products.
"""

import numpy as np

import concourse.bacc as bacc
import concourse.tile as tile
from concourse import mybir
from concourse.bass import ts, ds
from concourse.bass_utils import run_bass_kernel_spmd

F32 = mybir.dt.float32
F32R = mybir.dt.float32r

# problem geometry (hardcoded per contest rules)
S = 2048
DIM = 4096
HD = 128
N_HEADS = 32
N_KV = 8
NCORES = 8
HPC = N_HEADS // NCORES       # 4 q heads per core
FEAT = HPC * HD               # 512 per-core attention feature width

SBW = 256                     # QKV projection s-block width
NSB = S // SBW                # 8
KCH = DIM // 128              # 32 contraction chunks
QBW = 512                     # attention q-block width
NQB = S // QBW                # 4
NSC = S // 128                # 16 kv chunks
OBW = 512                     # output-dim block width
NOB = DIM // OBW              # 8

_CACHE = {}


def _build():
    nc = bacc.Bacc("TRN2", target_bir_lowering=False, debug=False,
                   num_devices=NCORES)

    xT = nc.dram_tensor("xT", [DIM, S], F32R, kind="ExternalInput").ap()
    wqT = nc.dram_tensor("wqT", [DIM, FEAT], F32R, kind="ExternalInput").ap()
    wkT = nc.dram_tensor("wkT", [DIM, HD], F32R, kind="ExternalInput").ap()
    wvT = nc.dram_tensor("wvT", [DIM, HD], F32R, kind="ExternalInput").ap()
    woT = nc.dram_tensor("woT", [FEAT, DIM], F32R, kind="ExternalInput").ap()
    cos2 = nc.dram_tensor("cos2", [128, S], F32, kind="ExternalInput").ap()
    sin2 = nc.dram_tensor("sin2", [128, S], F32, kind="ExternalInput").ap()
    gmask = nc.dram_tensor("gmask", [128, 896], F32, kind="ExternalInput").ap()
    onesc = nc.dram_tensor("onesc", [128, 1], F32R, kind="ExternalInput").ap()
    onesr = nc.dram_tensor("onesr", [1, 128], F32R, kind="ExternalInput").ap()
    sgn = nc.dram_tensor("sgn", [128, 1], F32, kind="ExternalInput").ap()
    out_d = nc.dram_tensor("out", [S, DIM], F32, kind="ExternalOutput").ap()

    with tile.TileContext(nc) as tc:
        with (
            tc.tile_pool(name="dram", bufs=1, space="DRAM") as dpool,
            tc.tile_pool(name="res", bufs=1) as res,          # KT + small consts
        ):
            qt_d = dpool.tile([FEAT, S], F32R, tag="qt")
            vt_d = dpool.tile([HD, S], F32R, tag="vt")

            kt_t = res.tile([128, S], F32R, tag="kt")
            onesc_t = res.tile([128, 1], F32R, tag="onesc")
            onesr_t = res.tile([1, 128], F32R, tag="onesr")
            gm_t = res.tile([128, 896], F32, tag="gm")
            sgn_t = res.tile([128, 1], F32, tag="sgn")
            nc.sync.dma_start(out=onesc_t, in_=onesc)
            nc.sync.dma_start(out=onesr_t, in_=onesr)
            nc.sync.dma_start(out=gm_t, in_=gmask)
            nc.sync.dma_start(out=sgn_t, in_=sgn)

            # ---------------- Phase 1: QKV projections + RoPE ----------------
            with (
                tc.tile_pool(name="wq", bufs=1) as wqp,
                tc.tile_pool(name="wkv", bufs=1) as wkvp,
                tc.tile_pool(name="xt", bufs=2) as xtp,
                tc.tile_pool(name="trig", bufs=1) as trigp,
                tc.tile_pool(name="rope", bufs=3) as ropep,
                tc.tile_pool(name="stage1", bufs=3) as st1p,
                tc.tile_pool(name="qkvps", bufs=3, space="PSUM") as qkvps,
            ):
                wq_t = wqp.tile([128, KCH, FEAT], F32R, tag="wq")
                nc.sync.dma_start(out=wq_t, in_=wqT.rearrange("(k p) m -> p k m", p=128))
                wk_t = wkvp.tile([128, KCH, HD], F32R, tag="wk")
                nc.sync.dma_start(out=wk_t, in_=wkT.rearrange("(k p) m -> p k m", p=128))
                wv_t = wkvp.tile([128, KCH, HD], F32R, tag="wv")
                nc.sync.dma_start(out=wv_t, in_=wvT.rearrange("(k p) m -> p k m", p=128))
                cos_t = trigp.tile([128, S], F32, tag="cos")
                nc.sync.dma_start(out=cos_t, in_=cos2)
                sin_t = trigp.tile([128, S], F32, tag="sin")
                nc.sync.dma_start(out=sin_t, in_=sin2)

                for sb in range(NSB):
                    xt_t = xtp.tile([128, KCH, SBW], F32R, tag="xt")
                    nc.sync.dma_start(
                        out=xt_t,
                        in_=xT[:, ts(sb, SBW)].rearrange("(k p) s -> p k s", p=128),
                    )
                    c_sl = cos_t[:, ts(sb, SBW)]
                    s_sl = sin_t[:, ts(sb, SBW)]
                    for ob in range(HPC + 2):  # 0..3 q heads, 4 = k, 5 = v
                        ps = qkvps.tile([128, SBW], F32, tag="ps")
                        for k in range(KCH):
                            if ob < HPC:
                                lhs = wq_t[:, k, ts(ob, HD)]
                            elif ob == HPC:
                                lhs = wk_t[:, k, :]
                            else:
                                lhs = wv_t[:, k, :]
                            nc.tensor.matmul(ps, lhs, xt_t[:, k, :],
                                             start=(k == 0), stop=(k == KCH - 1))
                        if ob <= HPC:
                            # RoPE (q heads and k): rot = (swap_halves(x*sin) * sgn) + x*cos
                            # where sgn = [-1]*64 + [+1]*64; the half swap is a DMA
                            # (DVE lanes cannot cross partitions).
                            m1 = ropep.tile([128, SBW], F32, tag="m1")
                            m2 = ropep.tile([128, SBW], F32, tag="m2")
                            nc.vector.tensor_mul(m1, ps, c_sl)
                            nc.vector.tensor_mul(m2, ps, s_sl)
                            w = ropep.tile([128, SBW], F32, tag="w")
                            nc.sync.dma_start(out=w[0:64], in_=m2[64:128])
                            nc.sync.dma_start(out=w[64:128], in_=m2[0:64])
                            if ob < HPC:
                                dst = st1p.tile([128, SBW], F32R, tag="qst")
                            else:
                                dst = kt_t[:, ts(sb, SBW)]
                            nc.vector.scalar_tensor_tensor(
                                dst, w, sgn_t, m1,
                                op0=mybir.AluOpType.mult, op1=mybir.AluOpType.add)
                            if ob < HPC:
                                nc.sync.dma_start(
                                    out=qt_d[ts(ob, HD), ts(sb, SBW)], in_=dst)
                        else:
                            vst = st1p.tile([128, SBW], F32R, tag="vst")
                            nc.scalar.copy(vst, ps)
                            nc.sync.dma_start(out=vt_d[:, ts(sb, SBW)], in_=vst)

            # ---------------- Phase 2: attention + output projection --------
            with (
                tc.tile_pool(name="wo", bufs=1) as wop,
                tc.tile_pool(name="vres", bufs=1) as vresp,
                tc.tile_pool(name="qt", bufs=3) as qtp,
                tc.tile_pool(name="exp", bufs=4) as expp,
                tc.tile_pool(name="sum", bufs=2) as sump,
                tc.tile_pool(name="outT", bufs=8) as outTp,
                tc.tile_pool(name="bc", bufs=2) as bcp,
                tc.tile_pool(name="small", bufs=4) as smallp,
                tc.tile_pool(name="stage2", bufs=3) as st2p,
                tc.tile_pool(name="scps", bufs=2, space="PSUM") as scps,
                tc.tile_pool(name="pvps", bufs=2, space="PSUM") as pvps,
                tc.tile_pool(name="lbps", bufs=2, space="PSUM") as lbps,
                tc.tile_pool(name="prps", bufs=2, space="PSUM") as prps,
            ):
                wo_t = wop.tile([128, HPC, DIM], F32R, tag="wo")
                nc.sync.dma_start(out=wo_t, in_=woT.rearrange("(h p) d -> p h d", p=128))
                v_t = vresp.tile([128, NSC, HD], F32R, tag="v")
                for sc in range(NSC):
                    nc.sync.dma_start(out=v_t[:, sc, :],
                                      in_=vt_d[:, ts(sc, 128)].rearrange("h s -> s h"))

                for qb in range(NQB):
                    outT_tiles = []
                    for h in range(HPC):
                        qt_t = qtp.tile([128, QBW], F32R, tag="qt")
                        nc.sync.dma_start(out=qt_t, in_=qt_d[ts(h, HD), ts(qb, QBW)])
                        pv_ps = pvps.tile([128, QBW], F32, tag="pv")
                        sum_t = sump.tile([128, QBW], F32R, tag="sum")
                        nsc = 4 * (qb + 1)
                        for sc in range(nsc):
                            s_ps = scps.tile([128, QBW], F32, tag="sc")
                            nc.tensor.matmul(s_ps, kt_t[:, ts(sc, 128)], qt_t,
                                             start=True, stop=True)
                            e_t = expp.tile([128, QBW], F32R, tag="exp")
                            nc.scalar.activation(e_t, s_ps,
                                                 mybir.ActivationFunctionType.Exp)
                            if sc >= 4 * qb:
                                t = sc - 4 * qb
                                nc.vector.tensor_mul(
                                    e_t, e_t, gm_t[:, ds(384 - 128 * t, QBW)])
                            if sc == 0:
                                nc.vector.tensor_copy(sum_t, e_t)
                            else:
                                nc.vector.tensor_add(sum_t, sum_t, e_t)
                            nc.tensor.matmul(pv_ps, v_t[:, sc, :], e_t,
                                             start=(sc == 0), stop=(sc == nsc - 1))
                        # per-query normalizer 1/l, broadcast across partitions
                        l_ps = lbps.tile([1, QBW], F32, tag="lb")
                        nc.tensor.matmul(l_ps, onesc_t, sum_t, start=True, stop=True)
                        recip = smallp.tile([1, QBW], F32, tag="recip")
                        nc.vector.reciprocal(recip, l_ps)
                        recip_r = smallp.tile([1, QBW], F32R, tag="recipr")
                        nc.vector.tensor_copy(recip_r, recip)
                        bc_ps = lbps.tile([128, QBW], F32, tag="lb")
                        nc.tensor.matmul(bc_ps, onesr_t, recip_r, start=True, stop=True)
                        bc_t = bcp.tile([128, QBW], F32, tag="bc")
                        nc.scalar.copy(bc_t, bc_ps)
                        outT_t = outTp.tile([128, QBW], F32R, tag="outT")
                        nc.vector.tensor_mul(outT_t, pv_ps, bc_t)
                        outT_tiles.append(outT_t)

                    # output projection for this q block
                    for qs in range(QBW // 128):
                        for ob in range(NOB):
                            p_ps = prps.tile([128, OBW], F32, tag="pr")
                            for h in range(HPC):
                                nc.tensor.matmul(p_ps,
                                                 outT_tiles[h][:, ts(qs, 128)],
                                                 wo_t[:, h, ts(ob, OBW)],
                                                 start=(h == 0), stop=(h == HPC - 1))
                            o_st = st2p.tile([128, OBW], F32, tag="ost")
                            nc.scalar.copy(o_st, p_ps)
                            nc.sync.dma_start(
                                out=out_d[ds(qb * QBW + qs * 128, 128), ts(ob, OBW)],
                                in_=o_st)

    nc.compile()
    return nc


def _host_prep(x, wq, wk, wv, wo, freqs_cos, freqs_sin):
    x = np.asarray(x, np.float32)
    wq = np.asarray(wq, np.float32)
    wk = np.asarray(wk, np.float32)
    wv = np.asarray(wv, np.float32)
    wo = np.asarray(wo, np.float32)
    cos = np.asarray(freqs_cos, np.float32)
    sin = np.asarray(freqs_sin, np.float32)

    scale = 1.0 / np.sqrt(np.float32(HD))
    perm = np.concatenate([np.arange(0, HD, 2), np.arange(1, HD, 2)])
    wq_p = (wq.reshape(N_HEADS, HD, DIM)[:, perm, :]).reshape(DIM, DIM) * scale
    wk_p = (wk.reshape(N_KV, HD, DIM)[:, perm, :]).reshape(N_KV * HD, DIM)

    xT = np.ascontiguousarray(x.reshape(S, DIM).T)
    cos2 = np.ascontiguousarray(np.concatenate([cos.T, cos.T], 0))
    sin2 = np.ascontiguousarray(np.concatenate([sin.T, sin.T], 0))
    G = (np.arange(128)[:, None] <= (np.arange(896)[None, :] - 384)).astype(np.float32)
    G = np.ascontiguousarray(G)
    onesc = np.ones((128, 1), np.float32)
    onesr = np.ones((1, 128), np.float32)
    sgnv = np.concatenate([-np.ones((64, 1), np.float32),
                           np.ones((64, 1), np.float32)])

    in_maps = []
    for c in range(NCORES):
        in_maps.append({
            "xT": xT,
            "wqT": np.ascontiguousarray(wq_p[c * FEAT:(c + 1) * FEAT].T),
            "wkT": np.ascontiguousarray(wk_p[c * HD:(c + 1) * HD].T),
            "wvT": np.ascontiguousarray(wv[c * HD:(c + 1) * HD].T),
            "woT": np.ascontiguousarray(wo[:, c * FEAT:(c + 1) * FEAT].T),
            "cos2": cos2,
            "sin2": sin2,
            "gmask": G,
            "onesc": onesc,
            "onesr": onesr,
            "sgn": sgnv,
        })
    return in_maps


def kernel(x, wq, wk, wv, wo, freqs_cos, freqs_sin, _trace=False):
    if "nc" not in _CACHE:
        _CACHE["nc"] = _build()
    nc = _CACHE["nc"]
    in_maps = _host_prep(x, wq, wk, wv, wo, freqs_cos, freqs_sin)
    res = run_bass_kernel_spmd(nc, in_maps, core_ids=list(range(NCORES)),
                               trace=_trace)
    _CACHE["last_result"] = res
    total = np.zeros((S, DIM), np.float64)
    for c in range(NCORES):
        total += res.results[c]["out"]
    return total.astype(np.float32).reshape(1, S, DIM)
